# revision 28
# baseline (speedup 1.0000x reference)
"""Trainium2 Bass kernel for nn_MiniMHCLM (moe_routing).

Strategy (8 NeuronCores, SPMD, no collectives):
  - vocab-sharded head matmul: core i holds w_head rows [i*VS:(i+1)*VS]
    (host-sliced, zero-padded to uniform VS) transposed to k-major bf16;
    it computes logits for all 4096 tokens x its vocab slice and the host
    concatenates along vocab.
  - token embeddings are pre-gathered AND pre-transposed on the host into
    xT [K, NT] bf16 (numerically identical to embed[ids].astype(bf16)),
    so the device pipeline runs fully k-major with zero PE transposes of x
    and no indirect DMA.
  - per-token coeffs: phi-stationary matmul gives logits^T [24, T] plus a
    ones-matmul row of sum(x^2); one small PE transpose per 128-token
    chunk moves both to token-major for the RMS scale, sigmoid/exp and
    the Sinkhorn iterations (DVE/ACT, batched per 512-token group).
  - mixing runs transposed: per-token coeffs are PE-transposed back to
    [24, T] and broadcast across partitions with stride-0 SBUF->SBUF
    DMAs; x_merge^T is then built with DVE/GPSIMD multiply-adds and fed
    straight into the head matmul as the stationary operand.
  - head matmul in bf16 with fp32 PSUM; PSUM evacuated by ACT/DVE copies
    to bf16 and DMA'd to DRAM bf16 (host converts to fp32).
  - software pipeline: group g's head matmuls overlap group g+1's coeff
    pipeline and group g+2's logits, keeping the PE stream dense.
"""

import numpy as np

HC, C, TMAX = 4, 256, 8
RMS_EPS, PRE_EPS, SINK_EPS, POST_MULT = 1e-6, 1e-4, 1e-6, 2.0
VOCAB = 50257
B, S = 2, 2048
K = HC * C            # 1024
M = HC * HC + 2 * HC  # 24
NKC = K // 128        # 8 k-chunks
NCORES = 8
NT = B * S            # 4096
VS = 6283             # vocab rows per core (8*6283 = 50264 >= 50257)
GT = 512              # tokens per group
NG = NT // GT         # 8 groups
NCH = GT // 128       # 4 chunks per group
VW = 512
NV = (VS + VW - 1) // VW          # 13 head tiles (12x512 + 139)


def _build():
    from contextlib import ExitStack
    from concourse import bass, bacc, mybir
    import concourse.tile as tile
    from concourse.masks import make_identity

    f32 = mybir.dt.float32
    bf16 = mybir.dt.bfloat16
    AX = mybir.AxisListType
    OP = mybir.AluOpType
    AF = mybir.ActivationFunctionType

    nc = bacc.Bacc(target_bir_lowering=False)
    xt_p = nc.declare_dram_parameter("xt", [K, NT], bf16, False)
    wvt_p = nc.declare_dram_parameter("wvt", [K, VS], bf16, False)
    wit_p = nc.declare_dram_parameter("wit", [C, C], bf16, False)
    phi_p = nc.declare_dram_parameter("phi", [K, M], bf16, False)
    b_p = nc.declare_dram_parameter("b", [1, M], f32, False)
    al_p = nc.declare_dram_parameter("al", [1, 3], f32, False)
    out_p = nc.declare_dram_parameter("out", [NT, VS], bf16, True)

    with ExitStack() as ctx:
        tc = ctx.enter_context(tile.TileContext(nc))
        const = ctx.enter_context(tc.tile_pool(name="const", bufs=1))
        wtp = ctx.enter_context(tc.tile_pool(name="wtp", bufs=1))
        xtp = ctx.enter_context(tc.tile_pool(name="xtp", bufs=3))
        lgp = ctx.enter_context(tc.tile_pool(name="lgp", bufs=2))
        cfp = ctx.enter_context(tc.tile_pool(name="cfp", bufs=2))
        plp = ctx.enter_context(tc.tile_pool(name="plp", bufs=1))
        mxp = ctx.enter_context(tc.tile_pool(name="mxp", bufs=2))
        wkp = ctx.enter_context(tc.tile_pool(name="wkp", bufs=4))
        x2p = ctx.enter_context(tc.tile_pool(name="x2p", bufs=1))
        stp = ctx.enter_context(tc.tile_pool(name="stp", bufs=5))
        psh = ctx.enter_context(tc.tile_pool(name="psh", bufs=3, space="PSUM"))
        psa = ctx.enter_context(tc.tile_pool(name="psa", bufs=1, space="PSUM"))
        psb = ctx.enter_context(tc.tile_pool(name="psb", bufs=1, space="PSUM"))
        pst = ctx.enter_context(tc.tile_pool(name="pst", bufs=2, space="PSUM"))
        psf = ctx.enter_context(tc.tile_pool(name="psf", bufs=1, space="PSUM"))
        drp = ctx.enter_context(tc.tile_pool(name="drp", bufs=2, space="DRAM"))

        # ---------------- constants ----------------
        ident = const.tile([128, 128], bf16)
        make_identity(nc, ident[:])
        identf = const.tile([128, 128], f32)
        make_identity(nc, identf[:])

        cst = const.tile([128, 2], f32)
        nc.vector.memset(cst[:, 0:1], 0.0)
        nc.vector.memset(cst[:, 1:2], RMS_EPS)
        zero_b = cst[:, 0:1]
        eps_b = cst[:, 1:2]

        ones = const.tile([128, 1], bf16)
        nc.vector.memset(ones[:], 1.0)

        phi_sb = const.tile([128, NKC * M], bf16)
        for kc in range(NKC):
            nc.sync.dma_start(out=phi_sb[:, kc * M:(kc + 1) * M],
                              in_=phi_p[kc * 128:(kc + 1) * 128, :])
        b_bc = const.tile([128, M], f32)
        nc.sync.dma_start(out=b_bc[:], in_=b_p[0:1, :].to_broadcast([128, M]))
        al_bc = const.tile([128, 3], f32)
        nc.sync.dma_start(out=al_bc[:], in_=al_p[0:1, :].to_broadcast([128, 3]))

        # w_inner^T (k-major [c, o]) as 2 row bands
        wit_sb = const.tile([128, 2 * C], bf16)
        for h in range(2):
            nc.sync.dma_start(out=wit_sb[:, h * C:(h + 1) * C],
                              in_=wit_p[h * 128:(h + 1) * 128, :])

        # w_head^T slice, 8 k row bands
        wt_all = wtp.tile([128, NKC * VS], bf16, tag="wt_all")
        for kc in range(NKC):
            nc.sync.dma_start(out=wt_all[:, kc * VS:(kc + 1) * VS],
                              in_=wvt_p[kc * 128:(kc + 1) * 128, :])

        st = {}  # per-group live tiles

        # ---------------- pipeline stages ----------------
        def stage_lg(g):
            """xT DMA, phi logits^T + sumsq row, transpose to token-major,
            RMS scale + coeff activations + Sinkhorn -> coefs [128, 4*24]."""
            t0 = g * GT
            xtg = xtp.tile([128, NKC * GT], bf16, tag="xtg", name=f"xtg{g}")
            for kc in range(NKC):
                nc.sync.dma_start(
                    out=xtg[:, kc * GT:(kc + 1) * GT],
                    in_=xt_p[kc * 128:(kc + 1) * 128, t0:t0 + GT])

            # squares for the RMS sum (bf16 is plenty for the mean)
            x2s = []
            for half in range(2):
                x2 = x2p.tile([128, 4 * GT], bf16, tag=f"x2{half}",
                              name=f"x2_{g}_{half}")
                sl = slice(half * 4 * GT, (half + 1) * 4 * GT)
                nc.gpsimd.tensor_tensor(
                    out=x2[:], in0=xtg[:, sl], in1=xtg[:, sl], op=OP.mult)
                x2s.append(x2)

            pslg = psa.tile([32, GT], f32, tag="pslg")
            psss = psb.tile([32, GT], f32, tag="psss")
            for kc in range(NKC):
                nc.tensor.matmul(
                    out=pslg[0:M, :],
                    lhsT=phi_sb[:, kc * M:(kc + 1) * M],
                    rhs=xtg[:, kc * GT:(kc + 1) * GT],
                    start=(kc == 0), stop=(kc == NKC - 1))
            for kc in range(NKC):
                nc.tensor.matmul(
                    out=psss[0:1, :],
                    lhsT=ones[:],
                    rhs=x2s[kc // 4][:, (kc % 4) * GT:(kc % 4 + 1) * GT],
                    start=(kc == 0), stop=(kc == NKC - 1))

            lgsb = lgp.tile([32, GT], f32, tag="lgsb", name=f"lgsb{g}")
            nc.scalar.copy(lgsb[0:M, :], pslg[0:M, :])
            ssq = lgp.tile([1, GT], f32, tag="ssq", name=f"ssq{g}")
            nc.scalar.copy(ssq[0:1, :], psss[0:1, :])

            # token-major [128, 4, 24] + per-token sumsq column
            lgtm = lgp.tile([128, NCH * 32], f32, tag="lgtm", name=f"lgtm{g}")
            msq = lgp.tile([128, NCH], f32, tag="msq", name=f"msq{g}")
            for tcx in range(NCH):
                pT = pst.tile([128, 128], f32, tag="psT")
                nc.tensor.transpose(
                    out=pT[:, 0:M],
                    in_=lgsb[0:M, tcx * 128:(tcx + 1) * 128],
                    identity=identf[0:M, 0:M])
                nc.tensor.transpose(
                    out=pT[:, M:M + 1],
                    in_=ssq[0:1, tcx * 128:(tcx + 1) * 128],
                    identity=identf[0:1, 0:1])
                nc.scalar.copy(lgtm[:, tcx * 32:tcx * 32 + M], pT[:, 0:M])
                nc.scalar.copy(msq[:, tcx:tcx + 1], pT[:, M:M + 1])
            lgv = lgtm[:].rearrange("p (c w) -> p c w", w=32)

            # scl = 1/sqrt(mean+eps)
            scl = lgp.tile([128, NCH], f32, tag="scl", name=f"scl{g}")
            nc.scalar.activation(out=scl[:], in_=msq[:],
                                 func=AF.Sqrt, scale=1.0 / K, bias=eps_b)
            nc.vector.reciprocal(scl[:], scl[:])
            for tcx in range(NCH):
                nc.vector.tensor_scalar_mul(
                    lgv[:, tcx, 0:M], lgv[:, tcx, 0:M], scl[:, tcx:tcx + 1])
            nc.vector.tensor_tensor(
                out=lgv[:, :, 0:M], in0=lgv[:, :, 0:M],
                in1=b_bc[:][:, None, :].to_broadcast([128, NCH, M]), op=OP.add)

            # coefs [128, 4, 24]: [0:16]=exp(res), [16:20]=h_pre, [20:24]=h_post2
            coefs = cfp.tile([128, NCH * M], f32, tag="coefs", name=f"coefs{g}")
            cfv = coefs[:].rearrange("p (c m) -> p c m", m=M)
            nc.scalar.activation(out=cfv[:, :, 16:20], in_=lgv[:, :, 0:4],
                                 func=AF.Sigmoid, bias=zero_b,
                                 scale=al_bc[:, 0:1])
            nc.vector.tensor_scalar_add(cfv[:, :, 16:20], cfv[:, :, 16:20],
                                        PRE_EPS)
            nc.scalar.activation(out=cfv[:, :, 20:24], in_=lgv[:, :, 4:8],
                                 func=AF.Sigmoid, bias=zero_b,
                                 scale=al_bc[:, 1:2])
            nc.vector.tensor_scalar_mul(cfv[:, :, 20:24], cfv[:, :, 20:24],
                                        POST_MULT)
            nc.scalar.activation(out=cfv[:, :, 0:16], in_=lgv[:, :, 8:24],
                                 func=AF.Exp, bias=zero_b, scale=al_bc[:, 2:3])

            # batched Sinkhorn on cfv[:, :, 0:16]
            mv4 = cfv[:, :, 0:16].rearrange("p c (o i) -> p c o i", i=4)
            mv4t = cfv[:, :, 0:16].rearrange("p c (o i) -> p c i o", i=4)
            # SINK_EPS (1e-6 vs O(1) row sums) is dropped: it shifts the
            # result by ~1e-6 relative, far below the bf16 noise floor.
            for _ in range(TMAX):
                rs = wkp.tile([128, NCH * 4], f32, tag="rs")
                rsv = rs[:].rearrange("p (c o) -> p c o", c=NCH)
                nc.vector.tensor_reduce(rsv, mv4, axis=AX.X, op=OP.add)
                nc.vector.reciprocal(rs[:], rs[:])
                nc.vector.tensor_tensor(
                    out=mv4, in0=mv4,
                    in1=rsv[:, :, :, None].to_broadcast([128, NCH, 4, 4]),
                    op=OP.mult)
                cs = wkp.tile([128, NCH * 4], f32, tag="cs")
                csv = cs[:].rearrange("p (c i) -> p c i", c=NCH)
                nc.vector.tensor_reduce(csv, mv4t, axis=AX.X, op=OP.add)
                nc.vector.reciprocal(cs[:], cs[:])
                nc.vector.tensor_tensor(
                    out=mv4, in0=mv4,
                    in1=csv[:, :, None, :].to_broadcast([128, NCH, 4, 4]),
                    op=OP.mult)
            st[g] = dict(xtg=xtg, coefs=coefs)

        def stage_planes(g):
            """Transpose coefs back to [24, T], broadcast each row across
            partitions (stride-0 SBUF->SBUF DMA) -> planes [128, 24*512]."""
            coefs = st[g]["coefs"]
            ctstg = cfp.tile([32, GT], bf16, tag="ctstg", name=f"ctstg{g}")
            for tcx in range(NCH):
                pT = pst.tile([128, 128], f32, tag="psT")
                nc.tensor.transpose(
                    out=pT[0:M, 0:128],
                    in_=coefs[:, tcx * M:(tcx + 1) * M],
                    identity=identf[:, 0:128])
                nc.scalar.copy(
                    ctstg[0:M, tcx * 128:(tcx + 1) * 128], pT[0:M, 0:128])
            dtile = drp.tile([1, M * GT], bf16, tag="cfdram",
                             name=f"cfdram{g}")
            nc.sync.dma_start(
                out=dtile[0:1, :].rearrange("x (c t) -> (x c) t", c=M),
                in_=ctstg[0:M, :])
            planes = plp.tile([128, M * GT], bf16, tag="planes",
                              name=f"planes{g}")
            nc.sync.dma_start(
                out=planes[:],
                in_=dtile[0:1, :].to_broadcast([128, M * GT]))
            st[g]["planes"] = planes
            # x_in^T = sum_i h_pre[i] * x^T[i]  (2 half-chunks of c)
            xtg = st[g]["xtg"]
            xin = mxp.tile([128, 2 * GT], bf16, tag="xin", name=f"xin{g}")
            for h in range(2):
                seg = xin[:, h * GT:(h + 1) * GT]
                nc.vector.tensor_tensor(
                    out=seg, in0=xtg[:, h * GT:(h + 1) * GT],
                    in1=planes[:, 16 * GT:17 * GT], op=OP.mult)
                for i in range(1, HC):
                    tmp = wkp.tile([128, GT], bf16, tag="tmp")
                    nc.vector.tensor_tensor(
                        out=tmp[:], in0=xtg[:, (i * 2 + h) * GT:
                                            (i * 2 + h + 1) * GT],
                        in1=planes[:, (16 + i) * GT:(17 + i) * GT],
                        op=OP.mult)
                    eng = nc.vector if i % 2 else nc.gpsimd
                    eng.tensor_add(seg, seg, tmp[:])
            st[g]["xin"] = xin

        def stage_fo(g):
            """f_out^T = w_inner @ x_in^T : 2 o-blocks x 2 c-halves."""
            xin = st[g]["xin"]
            fo = mxp.tile([128, 2 * GT], bf16, tag="fo", name=f"fo{g}")
            for ob in range(2):
                pf = psf.tile([128, GT], f32, tag="psf")
                for h in range(2):
                    nc.tensor.matmul(
                        out=pf[:],
                        lhsT=wit_sb[:, h * C + ob * 128:h * C + (ob + 1) * 128],
                        rhs=xin[:, h * GT:(h + 1) * GT],
                        start=(h == 0), stop=(h == 1))
                nc.scalar.copy(fo[:, ob * GT:(ob + 1) * GT], pf[:])
            st[g]["fo"] = fo

        def stage_mix(g):
            """x_merge^T[kc] = sum_i res[o,i]*x^T[i,h] + post2[o]*f_out^T[h]"""
            xtg, planes, fo = st[g]["xtg"], st[g]["planes"], st[g]["fo"]
            xmg = mxp.tile([128, NKC * GT], bf16, tag="xmg", name=f"xmg{g}")
            for kc in range(NKC):
                o, h = kc // 2, kc % 2
                seg = xmg[:, kc * GT:(kc + 1) * GT]
                nc.vector.tensor_tensor(
                    out=seg, in0=xtg[:, h * GT:(h + 1) * GT],
                    in1=planes[:, (o * 4) * GT:(o * 4 + 1) * GT], op=OP.mult)
                for i in range(1, HC):
                    tmp = wkp.tile([128, GT], bf16, tag="tmp")
                    nc.vector.tensor_tensor(
                        out=tmp[:],
                        in0=xtg[:, (i * 2 + h) * GT:(i * 2 + h + 1) * GT],
                        in1=planes[:, (o * 4 + i) * GT:(o * 4 + i + 1) * GT],
                        op=OP.mult)
                    eng = nc.vector if i % 2 else nc.gpsimd
                    eng.tensor_add(seg, seg, tmp[:])
                tmp = wkp.tile([128, GT], bf16, tag="tmp")
                nc.vector.tensor_tensor(
                    out=tmp[:], in0=fo[:, h * GT:(h + 1) * GT],
                    in1=planes[:, (20 + o) * GT:(21 + o) * GT], op=OP.mult)
                nc.gpsimd.tensor_add(seg, seg, tmp[:])
            st[g]["xmg"] = xmg

        def head_chunk(g, tcx):
            xmg = st[g]["xmg"]
            t0 = g * GT + tcx * 128
            stg = None
            for v in range(NV):
                w = min(VW, VS - v * VW)
                ph = psh.tile([128, VW], f32, tag="psh")
                for kc in range(NKC):
                    nc.tensor.matmul(
                        out=ph[:, 0:w],
                        lhsT=xmg[:, kc * GT + tcx * 128:
                                 kc * GT + (tcx + 1) * 128],
                        rhs=wt_all[:, kc * VS + v * VW:kc * VS + v * VW + w],
                        start=(kc == 0), stop=(kc == NKC - 1))
                # pair two v-tiles per staging tile / output DMA
                half = v % 2
                if half == 0:
                    stg = stp.tile([128, 2 * VW], bf16, tag="stg")
                nc.scalar.copy(stg[:, half * VW:half * VW + w], ph[:, 0:w])
                if half == 1 or v == NV - 1:
                    v0 = v - half
                    ww = min(2 * VW, VS - v0 * VW)
                    nc.sync.dma_start(
                        out=out_p[t0:t0 + 128, v0 * VW:v0 * VW + ww],
                        in_=stg[:, 0:ww])

        # ---------------- emission (software pipeline) ----------------
        stage_lg(0)
        stage_lg(1)
        stage_planes(0)
        stage_fo(0)
        stage_mix(0)
        for g in range(NG):
            head_chunk(g, 0)
            if g + 1 < NG:
                stage_planes(g + 1)
            head_chunk(g, 1)
            if g + 1 < NG:
                stage_fo(g + 1)
                stage_mix(g + 1)
            head_chunk(g, 2)
            if g + 2 < NG:
                stage_lg(g + 2)
            head_chunk(g, 3)
            del st[g]

    if not nc.is_finalized():
        nc.finalize()
    return nc


_NC_CACHE = {}


def _get_nc():
    if "nc" not in _NC_CACHE:
        _NC_CACHE["nc"] = _build()
    return _NC_CACHE["nc"]


def _make_in_maps(input_ids, embed, w_inner, w_head, phi, b,
                  alpha_pre, alpha_post, alpha_res):
    import ml_dtypes
    bf = ml_dtypes.bfloat16

    ids = np.asarray(input_ids).reshape(-1).astype(np.int64)
    x = np.asarray(embed)[ids].astype(bf)                 # [NT, K]
    xt = np.ascontiguousarray(x.T)                        # [K, NT]
    phi_np = np.ascontiguousarray(np.asarray(phi).astype(bf))
    wit = np.ascontiguousarray(np.asarray(w_inner).astype(bf).T)  # [c, o]
    b_np = np.ascontiguousarray(np.asarray(b, dtype=np.float32).reshape(1, M))
    al = np.array([[np.asarray(alpha_pre).reshape(-1)[0],
                    np.asarray(alpha_post).reshape(-1)[0],
                    np.asarray(alpha_res).reshape(-1)[0]]], dtype=np.float32)
    wh = np.asarray(w_head).astype(bf)                    # [VOCAB, K]

    in_maps = []
    for i in range(NCORES):
        sl = wh[i * VS:(i + 1) * VS]                      # [<=VS, K]
        wvt = np.zeros((K, VS), bf)
        wvt[:, :sl.shape[0]] = sl.T
        in_maps.append(dict(xt=xt, wvt=np.ascontiguousarray(wvt),
                            wit=wit, phi=phi_np, b=b_np, al=al))
    return in_maps


def _run(in_maps, trace=False):
    from concourse.bass_utils import run_bass_kernel_spmd
    nc = _get_nc()
    return run_bass_kernel_spmd(nc, in_maps, list(range(NCORES)), trace=trace)


def kernel(input_ids, embed, w_inner, w_head, phi, b,
           alpha_pre, alpha_post, alpha_res):
    in_maps = _make_in_maps(input_ids, embed, w_inner, w_head, phi, b,
                            alpha_pre, alpha_post, alpha_res)
    res = _run(in_maps).results
    out = np.concatenate([np.asarray(res[i]["out"]) for i in range(NCORES)],
                         axis=1)[:, :VOCAB]
    return np.ascontiguousarray(out.reshape(B, S, VOCAB).astype(np.float32))
